# revision 25
# baseline (speedup 1.0000x reference)
"""GCNConv layer on 8 Trainium2 NeuronCores (Bass/Tile).

Strategy (graph/data parallel, dst-sharded):
  - 8 cores, each owns N/8 destination nodes (blocks of 128).
  - Full nfeat (bf16) replicated to every core's HBM; per-core edges are
    gathered with dma_gather (SWDGE), so no collectives are needed.
  - Host (numpy) does index-side prep only: bucket edges by
    (core, dst-block, src-half), sort, pad each segment to a uniform
    tile count, build one-hot helper index arrays.
  - On device, per dst block: DVE builds all TPB dst one-hots in two
    wide is_equal ops; TensorE matmul-scatters gathered messages into
    PSUM (feature-major).  Edge-feature sums and degrees are pure index
    data, so the host ships a per-dst vocab-count matrix (cmat) and a
    broadcast 1/(deg+1) (rdegb): efeat = emb.T @ cmat via one matmul
    per block, then y = (nfeat + neigh) * rdegb and out = W.T @ y + b.
  - int16 gather indices only reach 32767, so nfeat is split into two
    N/2-row tables (lo/hi src halves gathered separately).

Outputs are produced feature-major [128, NPAD] per core and
transposed/concatenated on the host.
"""
import sys

if "/opt/trn_rl_repo" not in sys.path:
    sys.path.insert(0, "/opt/trn_rl_repo")

import numpy as np
import ml_dtypes

import concourse.bass as bass
import concourse.mybir as mybir
import concourse.tile as tile
from concourse import bacc
from concourse.bass_utils import run_bass_kernel_spmd

bf16 = mybir.dt.bfloat16
f32 = mybir.dt.float32
i16 = mybir.dt.int16
npbf = ml_dtypes.bfloat16

D = 128
M = 8                 # cores
CHUNK = 4             # dst blocks per gather call pair

_cache = {}
AMP = 1  # bench-only: repeat the compute body AMP times (amplified timing)
ABLATE = set()  # perf-model ablation flags
DEBUG_TAPS = False  # extra DRAM outputs per stage
GATHER_CALL_TILES = 8  # <=8 tiles (1024 idxs) keeps single_packet mode


def _build(T, N, npc, nblk, lens):
    """Build + compile the SPMD kernel for T tiles per segment.

    lens[blk][seg] = gather length (multiple of 16) for that group, the
    max edge count over the 8 cores -- identical across cores, so the
    SPMD program stays uniform while skipping most pad-slot payload.
    """
    key = (T, N, npc, nblk, lens, AMP)
    if key in _cache:
        return _cache[key]

    TPB = 2 * T                      # tiles per block
    NT = nblk * TPB                  # tiles per core
    NE_SLOTS = NT * 128              # edge slots per core
    SEG = T * 128                    # slots per segment
    npad = nblk * 128
    split = N // 2
    nchunks = (nblk + CHUNK - 1) // CHUNK

    nc = bacc.Bacc("TRN2", target_bir_lowering=False, debug=False)

    d_tabA = nc.dram_tensor("tabA", [split, D], bf16, kind="ExternalInput").ap()
    d_tabB = nc.dram_tensor("tabB", [N - split, D], bf16, kind="ExternalInput").ap()
    d_idx = nc.dram_tensor("idx", [128, NE_SLOTS // 16], i16, kind="ExternalInput").ap()
    d_dstrel = nc.dram_tensor("dstrel", [128, NT], bf16, kind="ExternalInput").ap()
    d_iota = nc.dram_tensor("iota", [128, 128], bf16, kind="ExternalInput").ap()
    d_emb = nc.dram_tensor("emb", [32, D], bf16, kind="ExternalInput").ap()
    d_cmat = nc.dram_tensor("cmat", [32, npad], bf16, kind="ExternalInput").ap()
    d_rdegb = nc.dram_tensor("rdegb", [128, npad], bf16, kind="ExternalInput").ap()
    d_nfT = nc.dram_tensor("nfT", [128, npad], bf16, kind="ExternalInput").ap()
    d_W = nc.dram_tensor("W", [D, D], bf16, kind="ExternalInput").ap()
    d_b = nc.dram_tensor("b", [D, 1], f32, kind="ExternalInput").ap()
    d_out = nc.dram_tensor("out", [D, npad], f32, kind="ExternalOutput").ap()
    if DEBUG_TAPS:
        d_cnt = nc.dram_tensor("dbg_cnt", [32, npad], f32, kind="ExternalOutput").ap()
        d_y = nc.dram_tensor("dbg_y", [D, npad], f32, kind="ExternalOutput").ap()
        d_fm = nc.dram_tensor("dbg_fm", [D, npad], f32, kind="ExternalOutput").ap()
        d_rdeg = nc.dram_tensor("dbg_rdeg", [D, npad], f32, kind="ExternalOutput").ap()

    with tile.TileContext(nc) as tc:
        with (
            tc.tile_pool(name="const", bufs=1) as cpool,
            tc.tile_pool(name="gather", bufs=3) as gpool,
            tc.tile_pool(name="oh", bufs=2) as ohpool,
            tc.tile_pool(name="ep", bufs=3) as eppool,
            tc.tile_pool(name="psum", bufs=2, space="PSUM") as ppool,
            tc.tile_pool(name="psum_out", bufs=2, space="PSUM") as popool,
        ):
            t_idx = cpool.tile([128, NE_SLOTS // 16], i16)
            nc.sync.dma_start(t_idx[:], d_idx[:])
            t_dstrel = cpool.tile([128, NT], bf16)
            nc.sync.dma_start(t_dstrel[:], d_dstrel[:])
            t_iota = cpool.tile([128, 128], bf16)
            nc.sync.dma_start(t_iota[:], d_iota[:])
            t_emb = cpool.tile([32, D], bf16)
            nc.sync.dma_start(t_emb[:], d_emb[:])
            t_cmat = cpool.tile([32, npad], bf16)
            nc.sync.dma_start(t_cmat[:], d_cmat[:])
            t_rdegb = cpool.tile([128, npad], bf16)
            nc.sync.dma_start(t_rdegb[:], d_rdegb[:])
            t_W = cpool.tile([D, D], bf16)
            nc.sync.dma_start(t_W[:], d_W[:])
            t_b = cpool.tile([D, 1], f32)
            nc.sync.dma_start(t_b[:], d_b[:])

            # dma_gather with single_packet=True is limited to 1024 indices;
            # one call per (block, src-half) group, trimmed to the group's
            # max-over-cores edge count. Tiles beyond the trimmed length keep
            # stale SBUF content, which the all-zero one-hot columns mask.
            def gather_group(g, tile0, tab, slot0, n16):
                if "gather" in ABLATE or n16 == 0:
                    return
                ntc = (n16 + 127) // 128
                nc.gpsimd.dma_gather(
                    g[:, tile0:tile0 + ntc, :], tab,
                    t_idx[:, slot0 // 16:slot0 // 16 + n16 // 16],
                    n16, n16, D,
                    single_packet=(n16 <= 1024),
                )

            for c in [cc for _rep in range(AMP) for cc in range(nchunks)]:
                nb = min(CHUNK, nblk - c * CHUNK)
                chunk_slot0 = c * CHUNK * TPB * 128  # first slot of chunk
                g = gpool.tile([128, CHUNK * TPB, 128], bf16, tag="g")
                if c < 3:
                    # first rotation of the pool: zero-init so untouched pad
                    # tiles can't hold NaN bit patterns (0*NaN != 0)
                    nc.vector.memset(g[:].rearrange("p a b -> p (a b)"), 0.0)
                for j in range(nb):
                    gather_group(g, j * T, d_tabA[:],
                                 chunk_slot0 + j * T * 128,
                                 lens[c * CHUNK + j][0])
                    gather_group(g, nb * T + j * T, d_tabB[:],
                                 chunk_slot0 + (nb * T + j * T) * 128,
                                 lens[c * CHUNK + j][1])
                blk0 = c * CHUNK
                wid = nb * 128
                psum_fm = ppool.tile([128, CHUNK * 128], f32, tag="fm")
                gtile0 = c * CHUNK * TPB
                oh = ohpool.tile([128, CHUNK * TPB, 128], bf16, tag="ohd")
                if "ohd" not in ABLATE:
                    # one wide build: all dst one-hots for this chunk
                    nc.vector.tensor_tensor(
                        out=oh[:, 0:nb * TPB, :],
                        in0=t_dstrel[:, gtile0:gtile0 + nb * TPB]
                            .rearrange("p (t o) -> p t o", o=1)
                            .to_broadcast([128, nb * TPB, 128]),
                        in1=t_iota[:].rearrange("p (o e) -> p o e", o=1)
                            .to_broadcast([128, nb * TPB, 128]),
                        op=mybir.AluOpType.is_equal,
                    )
                for j in range(nb):
                    blk = blk0 + j
                    fmj = psum_fm[:, j * 128:(j + 1) * 128]
                    for t in range(TPB):
                        seg, ts_ = (0, t) if t < T else (1, t - T)
                        slot = seg * nb * T + j * T + ts_
                        first = t == 0
                        if "mm" not in ABLATE:
                            nc.tensor.matmul(
                                out=fmj, lhsT=g[:, slot, :], rhs=oh[:, slot, :],
                                start=first, stop=False,
                            )
                    nc.tensor.matmul(
                        out=fmj, lhsT=t_emb[:],
                        rhs=t_cmat[:, blk * 128:(blk + 1) * 128],
                        start=False, stop=True,
                    )
                # chunk-wide epilogue over nb blocks at once
                nfT_ch = eppool.tile([128, CHUNK * 128], bf16, tag="nfT")
                nc.sync.dma_start(nfT_ch[:, :wid],
                                  d_nfT[:, blk0 * 128:blk0 * 128 + wid])
                ysum = eppool.tile([128, CHUNK * 128], f32, tag="ysum")
                nc.vector.tensor_tensor(
                    out=ysum[:, :wid], in0=psum_fm[:, :wid], in1=nfT_ch[:, :wid],
                    op=mybir.AluOpType.add,
                )
                y = eppool.tile([128, CHUNK * 128], bf16, tag="y")
                nc.vector.tensor_tensor(
                    out=y[:, :wid], in0=ysum[:, :wid],
                    in1=t_rdegb[:, blk0 * 128:blk0 * 128 + wid],
                    op=mybir.AluOpType.mult,
                )
                psum_out = popool.tile([128, CHUNK * 128], f32, tag="po")
                nc.tensor.matmul(
                    out=psum_out[:, :wid], lhsT=t_W[:], rhs=y[:, :wid],
                    start=True, stop=True,
                )
                out_sb = eppool.tile([128, CHUNK * 128], f32, tag="osb")
                nc.vector.tensor_scalar_add(out_sb[:, :wid], psum_out[:, :wid],
                                            t_b[:, 0:1])
                nc.sync.dma_start(
                    d_out[:, blk0 * 128:blk0 * 128 + wid], out_sb[:, :wid]
                )

    nc.compile()
    _cache[key] = nc
    return nc


def prepare(nfeat, src, dst, efeat_idx, edge_emb, W, b):
    """Host-side prep: returns (nc, in_maps, assembler)."""
    nfeat = np.asarray(nfeat, np.float32)
    src = np.asarray(src, np.int64)
    dst = np.asarray(dst, np.int64)
    efeat_idx = np.asarray(efeat_idx, np.int64)
    edge_emb = np.asarray(edge_emb, np.float32)
    W = np.asarray(W, np.float32)
    b = np.asarray(b, np.float32)

    N, _ = nfeat.shape
    E = src.shape[0]
    NF, V, _ = edge_emb.shape
    npc = N // M
    nblk = (npc + 127) // 128
    npad = nblk * 128
    split = N // 2

    core = dst // npc
    dst_local = dst % npc
    blk = dst_local // 128
    rel = (dst_local % 128).astype(np.float32)
    seg = (src >= split).astype(np.int64)

    # group id = ((core*nblk + blk)*2 + seg); rank of edge within group
    gid = (core * nblk + blk) * 2 + seg
    order = np.argsort(gid, kind="stable")
    gsorted = gid[order]
    counts = np.bincount(gid, minlength=M * nblk * 2)
    starts = np.concatenate([[0], np.cumsum(counts)[:-1]])
    rank = np.empty(E, np.int64)
    rank[order] = np.arange(E) - starts[gsorted]

    T = max(1, int((counts.max() + 127) // 128))
    TPB = 2 * T
    NT = nblk * TPB
    NE_SLOTS = NT * 128

    # slot of each edge within its core's slot space
    c_of_blk = blk // CHUNK
    j_of_blk = blk % CHUNK
    nb_of_blk = np.minimum(CHUNK, nblk - c_of_blk * CHUNK)
    chunk_slot0 = c_of_blk * CHUNK * TPB * 128
    slot = chunk_slot0 + (seg * nb_of_blk * T + j_of_blk * T) * 128 + rank

    # per-core packed arrays
    idx_all = np.zeros((M, NE_SLOTS), np.int16)
    dstrel_all = np.full((M, NE_SLOTS), -1.0, np.float32)
    idx_all[core, slot] = (src - seg * split).astype(np.int16)
    dstrel_all[core, slot] = rel

    # host-computed per-dst count matrix (vocab slots 8..31) and 1/(deg+1)
    dst_local_pad = core * npad + blk * 128 + (dst_local % 128)
    cmat_all = np.zeros((32, M * npad), np.float32)
    for c_ in range(NF):
        np.add.at(cmat_all, (8 + c_ * V + efeat_idx[:, c_], dst_local_pad), 1.0)
    deg_all = np.zeros(M * npad, np.float32)
    np.add.at(deg_all, dst_local_pad, 1.0)
    rdeg_all = 1.0 / (deg_all + 1.0)

    nfeat_bf = nfeat.astype(npbf)
    tabA = np.ascontiguousarray(nfeat_bf[:split])
    tabB = np.ascontiguousarray(nfeat_bf[split:])
    iota_b = np.tile(np.arange(128, dtype=np.float32)[None, :], (128, 1)).astype(npbf)
    emb32 = np.zeros((32, D), np.float32)
    emb32[8:8 + NF * V] = edge_emb.reshape(NF * V, D)
    emb32 = emb32.astype(npbf)
    W_bf = W.astype(npbf)
    b_col = b.reshape(D, 1).astype(np.float32)

    in_maps = []
    for k in range(M):
        idx_w = np.tile(
            np.ascontiguousarray(idx_all[k].reshape(NE_SLOTS // 16, 16).T), (8, 1)
        )
        dstrelT = np.ascontiguousarray(
            dstrel_all[k].reshape(NT, 128).T
        ).astype(npbf)
        nfT = np.zeros((128, npad), npbf)
        nfT[:, :npc] = nfeat_bf[k * npc:(k + 1) * npc].T
        cmat_k = np.ascontiguousarray(
            cmat_all[:, k * npad:(k + 1) * npad]).astype(npbf)
        rdegb_k = np.ascontiguousarray(np.tile(
            rdeg_all[k * npad:(k + 1) * npad][None, :], (128, 1))).astype(npbf)
        in_maps.append({
            "tabA": tabA, "tabB": tabB, "idx": idx_w, "dstrel": dstrelT,
            "iota": iota_b, "emb": emb32, "cmat": cmat_k, "rdegb": rdegb_k,
            "nfT": np.ascontiguousarray(nfT), "W": W_bf, "b": b_col,
        })

    # per-(block, src-half) gather length: max edge count over cores,
    # rounded up to the 16-index descriptor-lane granularity
    gmax = counts.reshape(M, nblk, 2).max(axis=0)
    lens = tuple(
        (int(-(-int(gmax[b_, 0]) // 16) * 16), int(-(-int(gmax[b_, 1]) // 16) * 16))
        for b_ in range(nblk)
    )

    nc = _build(T, N, npc, nblk, lens)

    def assemble(results):
        out = np.empty((N, D), np.float32)
        for k in range(M):
            out[k * npc:(k + 1) * npc] = results[k]["out"][:, :npc].T
        return out

    return nc, in_maps, assemble


def kernel(nfeat, src, dst, efeat_idx, edge_emb, W, b):
    nc, in_maps, assemble = prepare(nfeat, src, dst, efeat_idx, edge_emb, W, b)
    res = run_bass_kernel_spmd(nc, in_maps, core_ids=list(range(M)))
    return assemble(res.results)



# revision 26
# speedup vs baseline: 2.8005x; 2.8005x over previous
"""GCNConv layer on 8 Trainium2 NeuronCores (Bass/Tile).

Strategy (graph/data parallel, dst-sharded):
  - 8 cores, each owns N/8 destination nodes (blocks of 128).
  - Full nfeat (bf16) replicated to every core's HBM; per-core edges are
    gathered with dma_gather (SWDGE), so no collectives are needed.
  - Host (numpy) does index-side prep only: bucket edges by
    (core, dst-block, src-half), sort, pad each segment to a uniform
    tile count, build one-hot helper index arrays.
  - On device, per 4-block chunk: DVE builds every dst one-hot in one
    wide is_equal op; TensorE matmul-scatters gathered messages into a
    slice-chained PSUM region (feature-major).  Edge-feature sums and
    degrees are pure index data, so the host ships a per-dst vocab-count
    matrix (cmat) and a broadcast 1/(deg+1) (rdegb): efeat = emb.T @
    cmat via one matmul per block, then one chunk-wide epilogue computes
    y = (nfeat + neigh) * rdegb and out = W.T @ y + b.  Gather calls are
    trimmed per (block, src-half) group to the max-over-cores edge count
    so padding costs almost no gather payload.
  - int16 gather indices only reach 32767, so nfeat is split into two
    N/2-row tables (lo/hi src halves gathered separately).

Outputs are produced feature-major [128, NPAD] per core and
transposed/concatenated on the host.
"""
import sys

if "/opt/trn_rl_repo" not in sys.path:
    sys.path.insert(0, "/opt/trn_rl_repo")

import numpy as np
import ml_dtypes

import concourse.bass as bass
import concourse.mybir as mybir
import concourse.tile as tile
from concourse import bacc
from concourse.bass_utils import run_bass_kernel_spmd

bf16 = mybir.dt.bfloat16
f32 = mybir.dt.float32
i16 = mybir.dt.int16
npbf = ml_dtypes.bfloat16

D = 128
M = 8                 # cores
CHUNK = 4             # dst blocks per gather call pair

_cache = {}
AMP = 1  # bench-only: repeat the compute body AMP times (amplified timing)
ABLATE = set()  # perf-model ablation flags
DEBUG_TAPS = False  # extra DRAM outputs per stage
GATHER_CALL_TILES = 8  # <=8 tiles (1024 idxs) keeps single_packet mode


def _build(T, N, npc, nblk, lens):
    """Build + compile the SPMD kernel for T tiles per segment.

    lens[blk][seg] = gather length (multiple of 16) for that group, the
    max edge count over the 8 cores -- identical across cores, so the
    SPMD program stays uniform while skipping most pad-slot payload.
    """
    key = (T, N, npc, nblk, lens, AMP)
    if key in _cache:
        return _cache[key]

    TPB = 2 * T                      # tiles per block
    NT = nblk * TPB                  # tiles per core
    NE_SLOTS = NT * 128              # edge slots per core
    SEG = T * 128                    # slots per segment
    npad = nblk * 128
    split = N // 2
    nchunks = (nblk + CHUNK - 1) // CHUNK

    nc = bacc.Bacc("TRN2", target_bir_lowering=False, debug=False)

    d_tabA = nc.dram_tensor("tabA", [split, D], bf16, kind="ExternalInput").ap()
    d_tabB = nc.dram_tensor("tabB", [N - split, D], bf16, kind="ExternalInput").ap()
    d_idx = nc.dram_tensor("idx", [128, NE_SLOTS // 16], i16, kind="ExternalInput").ap()
    d_dstrel = nc.dram_tensor("dstrel", [128, NT], bf16, kind="ExternalInput").ap()
    d_iota = nc.dram_tensor("iota", [128, 128], bf16, kind="ExternalInput").ap()
    d_emb = nc.dram_tensor("emb", [32, D], bf16, kind="ExternalInput").ap()
    d_cmat = nc.dram_tensor("cmat", [32, npad], bf16, kind="ExternalInput").ap()
    d_rdegb = nc.dram_tensor("rdegb", [128, npad], bf16, kind="ExternalInput").ap()
    d_nfT = nc.dram_tensor("nfT", [128, npad], bf16, kind="ExternalInput").ap()
    d_W = nc.dram_tensor("W", [D, D], bf16, kind="ExternalInput").ap()
    d_b = nc.dram_tensor("b", [D, 1], f32, kind="ExternalInput").ap()
    d_out = nc.dram_tensor("out", [D, npad], f32, kind="ExternalOutput").ap()
    if DEBUG_TAPS:
        d_cnt = nc.dram_tensor("dbg_cnt", [32, npad], f32, kind="ExternalOutput").ap()
        d_y = nc.dram_tensor("dbg_y", [D, npad], f32, kind="ExternalOutput").ap()
        d_fm = nc.dram_tensor("dbg_fm", [D, npad], f32, kind="ExternalOutput").ap()
        d_rdeg = nc.dram_tensor("dbg_rdeg", [D, npad], f32, kind="ExternalOutput").ap()

    with tile.TileContext(nc) as tc:
        with (
            tc.tile_pool(name="const", bufs=1) as cpool,
            tc.tile_pool(name="gather", bufs=3) as gpool,
            tc.tile_pool(name="oh", bufs=2) as ohpool,
            tc.tile_pool(name="ep", bufs=3) as eppool,
            tc.tile_pool(name="psum", bufs=2, space="PSUM") as ppool,
            tc.tile_pool(name="psum_out", bufs=2, space="PSUM") as popool,
        ):
            t_idx = cpool.tile([128, NE_SLOTS // 16], i16)
            nc.sync.dma_start(t_idx[:], d_idx[:])
            t_dstrel = cpool.tile([128, NT], bf16)
            nc.sync.dma_start(t_dstrel[:], d_dstrel[:])
            t_iota = cpool.tile([128, 128], bf16)
            nc.sync.dma_start(t_iota[:], d_iota[:])
            t_emb = cpool.tile([32, D], bf16)
            nc.sync.dma_start(t_emb[:], d_emb[:])
            t_cmat = cpool.tile([32, npad], bf16)
            nc.sync.dma_start(t_cmat[:], d_cmat[:])
            t_rdegb = cpool.tile([128, npad], bf16)
            nc.sync.dma_start(t_rdegb[:], d_rdegb[:])
            t_W = cpool.tile([D, D], bf16)
            nc.sync.dma_start(t_W[:], d_W[:])
            t_b = cpool.tile([D, 1], f32)
            nc.sync.dma_start(t_b[:], d_b[:])

            # dma_gather with single_packet=True is limited to 1024 indices;
            # one call per (block, src-half) group, trimmed to the group's
            # max-over-cores edge count. Tiles beyond the trimmed length keep
            # stale SBUF content, which the all-zero one-hot columns mask.
            def gather_group(g, tile0, tab, slot0, n16):
                if "gather" in ABLATE or n16 == 0:
                    return
                ntc = (n16 + 127) // 128
                nc.gpsimd.dma_gather(
                    g[:, tile0:tile0 + ntc, :], tab,
                    t_idx[:, slot0 // 16:slot0 // 16 + n16 // 16],
                    n16, n16, D,
                    single_packet=(n16 <= 1024),
                )

            for c in [cc for _rep in range(AMP) for cc in range(nchunks)]:
                nb = min(CHUNK, nblk - c * CHUNK)
                chunk_slot0 = c * CHUNK * TPB * 128  # first slot of chunk
                g = gpool.tile([128, CHUNK * TPB, 128], bf16, tag="g")
                if c < 3:
                    # first rotation of the pool: zero-init so untouched pad
                    # tiles can't hold NaN bit patterns (0*NaN != 0)
                    nc.vector.memset(g[:].rearrange("p a b -> p (a b)"), 0.0)
                for j in range(nb):
                    gather_group(g, j * T, d_tabA[:],
                                 chunk_slot0 + j * T * 128,
                                 lens[c * CHUNK + j][0])
                    gather_group(g, nb * T + j * T, d_tabB[:],
                                 chunk_slot0 + (nb * T + j * T) * 128,
                                 lens[c * CHUNK + j][1])
                blk0 = c * CHUNK
                wid = nb * 128
                psum_fm = ppool.tile([128, CHUNK * 128], f32, tag="fm")
                gtile0 = c * CHUNK * TPB
                oh = ohpool.tile([128, CHUNK * TPB, 128], bf16, tag="ohd")
                if "ohd" not in ABLATE:
                    # one wide build: all dst one-hots for this chunk
                    nc.vector.tensor_tensor(
                        out=oh[:, 0:nb * TPB, :],
                        in0=t_dstrel[:, gtile0:gtile0 + nb * TPB]
                            .rearrange("p (t o) -> p t o", o=1)
                            .to_broadcast([128, nb * TPB, 128]),
                        in1=t_iota[:].rearrange("p (o e) -> p o e", o=1)
                            .to_broadcast([128, nb * TPB, 128]),
                        op=mybir.AluOpType.is_equal,
                    )
                for j in range(nb):
                    blk = blk0 + j
                    fmj = psum_fm[:, j * 128:(j + 1) * 128]
                    for t in range(TPB):
                        seg, ts_ = (0, t) if t < T else (1, t - T)
                        slot = seg * nb * T + j * T + ts_
                        first = t == 0
                        if "mm" not in ABLATE:
                            nc.tensor.matmul(
                                out=fmj, lhsT=g[:, slot, :], rhs=oh[:, slot, :],
                                start=first, stop=False,
                            )
                    nc.tensor.matmul(
                        out=fmj, lhsT=t_emb[:],
                        rhs=t_cmat[:, blk * 128:(blk + 1) * 128],
                        start=False, stop=True,
                    )
                # chunk-wide epilogue over nb blocks at once
                nfT_ch = eppool.tile([128, CHUNK * 128], bf16, tag="nfT")
                nc.sync.dma_start(nfT_ch[:, :wid],
                                  d_nfT[:, blk0 * 128:blk0 * 128 + wid])
                ysum = eppool.tile([128, CHUNK * 128], f32, tag="ysum")
                nc.vector.tensor_tensor(
                    out=ysum[:, :wid], in0=psum_fm[:, :wid], in1=nfT_ch[:, :wid],
                    op=mybir.AluOpType.add,
                )
                y = eppool.tile([128, CHUNK * 128], bf16, tag="y")
                nc.vector.tensor_tensor(
                    out=y[:, :wid], in0=ysum[:, :wid],
                    in1=t_rdegb[:, blk0 * 128:blk0 * 128 + wid],
                    op=mybir.AluOpType.mult,
                )
                psum_out = popool.tile([128, CHUNK * 128], f32, tag="po")
                nc.tensor.matmul(
                    out=psum_out[:, :wid], lhsT=t_W[:], rhs=y[:, :wid],
                    start=True, stop=True,
                )
                out_sb = eppool.tile([128, CHUNK * 128], f32, tag="osb")
                nc.vector.tensor_scalar_add(out_sb[:, :wid], psum_out[:, :wid],
                                            t_b[:, 0:1])
                nc.sync.dma_start(
                    d_out[:, blk0 * 128:blk0 * 128 + wid], out_sb[:, :wid]
                )

    nc.compile()
    _cache[key] = nc
    return nc


def prepare(nfeat, src, dst, efeat_idx, edge_emb, W, b):
    """Host-side prep: returns (nc, in_maps, assembler)."""
    nfeat = np.asarray(nfeat, np.float32)
    src = np.asarray(src, np.int64)
    dst = np.asarray(dst, np.int64)
    efeat_idx = np.asarray(efeat_idx, np.int64)
    edge_emb = np.asarray(edge_emb, np.float32)
    W = np.asarray(W, np.float32)
    b = np.asarray(b, np.float32)

    N, _ = nfeat.shape
    E = src.shape[0]
    NF, V, _ = edge_emb.shape
    npc = N // M
    nblk = (npc + 127) // 128
    npad = nblk * 128
    split = N // 2

    core = dst // npc
    dst_local = dst % npc
    blk = dst_local // 128
    rel = (dst_local % 128).astype(np.float32)
    seg = (src >= split).astype(np.int64)

    # group id = ((core*nblk + blk)*2 + seg); rank of edge within group
    gid = (core * nblk + blk) * 2 + seg
    order = np.argsort(gid, kind="stable")
    gsorted = gid[order]
    counts = np.bincount(gid, minlength=M * nblk * 2)
    starts = np.concatenate([[0], np.cumsum(counts)[:-1]])
    rank = np.empty(E, np.int64)
    rank[order] = np.arange(E) - starts[gsorted]

    T = max(1, int((counts.max() + 127) // 128))
    TPB = 2 * T
    NT = nblk * TPB
    NE_SLOTS = NT * 128

    # slot of each edge within its core's slot space
    c_of_blk = blk // CHUNK
    j_of_blk = blk % CHUNK
    nb_of_blk = np.minimum(CHUNK, nblk - c_of_blk * CHUNK)
    chunk_slot0 = c_of_blk * CHUNK * TPB * 128
    slot = chunk_slot0 + (seg * nb_of_blk * T + j_of_blk * T) * 128 + rank

    # per-core packed arrays
    idx_all = np.zeros((M, NE_SLOTS), np.int16)
    dstrel_all = np.full((M, NE_SLOTS), -1.0, np.float32)
    idx_all[core, slot] = (src - seg * split).astype(np.int16)
    dstrel_all[core, slot] = rel

    # host-computed per-dst count matrix (vocab slots 8..31) and 1/(deg+1)
    dst_local_pad = core * npad + blk * 128 + (dst_local % 128)
    cmat_all = np.zeros((32, M * npad), np.float32)
    for c_ in range(NF):
        np.add.at(cmat_all, (8 + c_ * V + efeat_idx[:, c_], dst_local_pad), 1.0)
    deg_all = np.zeros(M * npad, np.float32)
    np.add.at(deg_all, dst_local_pad, 1.0)
    rdeg_all = 1.0 / (deg_all + 1.0)

    nfeat_bf = nfeat.astype(npbf)
    tabA = np.ascontiguousarray(nfeat_bf[:split])
    tabB = np.ascontiguousarray(nfeat_bf[split:])
    iota_b = np.tile(np.arange(128, dtype=np.float32)[None, :], (128, 1)).astype(npbf)
    emb32 = np.zeros((32, D), np.float32)
    emb32[8:8 + NF * V] = edge_emb.reshape(NF * V, D)
    emb32 = emb32.astype(npbf)
    W_bf = W.astype(npbf)
    b_col = b.reshape(D, 1).astype(np.float32)

    in_maps = []
    for k in range(M):
        idx_w = np.tile(
            np.ascontiguousarray(idx_all[k].reshape(NE_SLOTS // 16, 16).T), (8, 1)
        )
        dstrelT = np.ascontiguousarray(
            dstrel_all[k].reshape(NT, 128).T
        ).astype(npbf)
        nfT = np.zeros((128, npad), npbf)
        nfT[:, :npc] = nfeat_bf[k * npc:(k + 1) * npc].T
        cmat_k = np.ascontiguousarray(
            cmat_all[:, k * npad:(k + 1) * npad]).astype(npbf)
        rdegb_k = np.ascontiguousarray(np.tile(
            rdeg_all[k * npad:(k + 1) * npad][None, :], (128, 1))).astype(npbf)
        in_maps.append({
            "tabA": tabA, "tabB": tabB, "idx": idx_w, "dstrel": dstrelT,
            "iota": iota_b, "emb": emb32, "cmat": cmat_k, "rdegb": rdegb_k,
            "nfT": np.ascontiguousarray(nfT), "W": W_bf, "b": b_col,
        })

    # per-(block, src-half) gather length: max edge count over cores,
    # rounded up to the 16-index descriptor-lane granularity
    gmax = counts.reshape(M, nblk, 2).max(axis=0)
    lens = tuple(
        (int(-(-int(gmax[b_, 0]) // 16) * 16), int(-(-int(gmax[b_, 1]) // 16) * 16))
        for b_ in range(nblk)
    )

    nc = _build(T, N, npc, nblk, lens)

    def assemble(results):
        out = np.empty((N, D), np.float32)
        for k in range(M):
            out[k * npc:(k + 1) * npc] = results[k]["out"][:, :npc].T
        return out

    return nc, in_maps, assemble


def kernel(nfeat, src, dst, efeat_idx, edge_emb, W, b):
    nc, in_maps, assemble = prepare(nfeat, src, dst, efeat_idx, edge_emb, W, b)
    res = run_bass_kernel_spmd(nc, in_maps, core_ids=list(range(M)))
    return assemble(res.results)

